# revision 23
# baseline (speedup 1.0000x reference)
"""GSPN Trainium2 kernel: batch x channel-half sharding over 8 cores.

Core c = 2*b + half.  Each core computes batch b; channels are split in
half through the middle of the network (gates/scan are channel-
independent).  Cross-core data exchange (pairwise within each batch):
  - AllGather of the 48-dim xdown projection partials (bf16) + local add
  - per-round AllGather of the merged scan output out_m (bf16), hidden
    under the scan of later rounds; outconv then consumes all 768 input
    channels locally, so no ReduceScatter is needed after outconv
  - ReduceScatter (bf16) of the outproj partial sums over pixel halves
Output per core: pixel rows [half*512 : half*512+512] of (1024, 768), bf16.
"""

import numpy as np
import ml_dtypes

import concourse.bass as bass
import concourse.mybir as mybir
import concourse.tile as tile
from concourse import bacc, bass_utils

F32 = mybir.dt.float32
BF16 = mybir.dt.bfloat16
AF = mybir.ActivationFunctionType
OP = mybir.AluOpType

D = 768
DH = 384          # channels per half
P = 1024          # pixels
HW = 32
CT = 3            # channel tiles per half
SCT = 12          # scan channel tiles (4 dirs x 3)
NR = 4            # scan rounds (one 8-col chunk each)
CW = 8            # columns per chunk
EPS = 1e-5

REPLICA_PAIRS = [[0, 1], [2, 3], [4, 5], [6, 7]]


def build_program():
    nc = bacc.Bacc("TRN2", target_bir_lowering=False, debug=False,
                   enable_asserts=True, num_devices=8)

    def din(name, shape, dt):
        return nc.dram_tensor(name, shape, dt, kind="ExternalInput").ap()

    hs = din("hs", [P, D], F32)
    w_in = din("w_in", [128, 6 * DH], BF16)       # in_proj lhsT, (ktile, M) packed
    b_in = din("b_in", [1, DH], BF16)
    dw7 = din("dw7", [CT, 128, 49 * 128], BF16)   # diag taps per ctile
    w7v = din("w7v", [128, CT * 49], BF16)        # per-channel tap values (DVE taps)
    b7 = din("b7", [1, DH], BF16)
    w_xd = din("w_xd", [128, CT * 48], BF16)
    w_g = din("w_g", [128, 72 * 128], BF16)       # rows 0:48
    w_oc = din("w_oc", [128, 6 * DH], BF16)       # 6 in-ktiles x 3 own-out mtiles
    dw3 = din("dw3", [128, CT * 9 * 128], BF16)
    w_op = din("w_op", [128, CT * D], BF16)
    ident = din("ident", [128, 128], BF16)
    y_out = nc.dram_tensor("y", [P // 2, D], BF16, kind="ExternalOutput").ap()

    with tile.TileContext(nc) as tc:
        with tc.tile_pool(name="wp", bufs=1) as wp, \
             tc.tile_pool(name="mid", bufs=1) as mid, \
             tc.tile_pool(name="dram", bufs=1, space="DRAM") as dramp:

            # ---- persistent weight tiles (DMAs issued near first use so the
            # serialized DMA pipeline services the hs load first) ----
            w_in_sb = wp.tile([128, 6 * DH], BF16, tag="w_in")
            b_in_sb = wp.tile([1, DH], BF16, tag="b_in")
            b7_sb = wp.tile([1, DH], BF16, tag="b7")
            w_xd_sb = wp.tile([128, CT * 48], BF16, tag="w_xd")
            w7v_sb = wp.tile([128, CT * 49], BF16, tag="w7v")
            w_g_sb = wp.tile([128, 72 * 128], BF16, tag="w_g")
            w_oc_sb = wp.tile([128, 6 * DH], BF16, tag="w_oc")
            dw3_sb = wp.tile([128, CT * 9 * 128], BF16, tag="dw3")
            w_op_sb = wp.tile([128, CT * D], BF16, tag="w_op")
            ident_sb = wp.tile([128, 128], BF16, tag="ident")
            ones_sb = wp.tile([1, 512], BF16, tag="ones")
            nc.vector.memset(ones_sb[:], 1.0)
            cst = wp.tile([128, 3], F32, tag="cst")
            nc.gpsimd.memset(cst[:, 0:1], EPS)
            nc.gpsimd.memset(cst[:, 1:2], -0.5)
            nc.gpsimd.memset(cst[:, 2:3], -1.0)
            # preload the sigmoid activation-table set during S0 so the first
            # gate eviction after the xp AllGather doesn't pay the table load
            warm = wp.tile([128, 1], F32, tag="warm")
            nc.scalar.activation(warm[:], cst[:, 0:1], AF.Sigmoid)

            # ---- mid-lifetime tensors ----
            x2h = mid.tile([128, CT * P], BF16, tag="x2h")   # (ct, h, w)
            x2w = mid.tile([128, CT * P], BF16, tag="x2w")   # (ct, w, h)
            xpb = mid.tile([48, P], BF16, tag="xpb")         # summed xdown, (h, w)
            out_m = mid.tile([128, CT * P], BF16, tag="out_m")  # (ct, w, h)

            x2h4 = x2h[:].rearrange("p (c h w) -> p c h w", c=CT, h=HW, w=HW)
            x2w4 = x2w[:].rearrange("p (c w h) -> p c w h", c=CT, w=HW, h=HW)
            out_m4 = out_m[:].rearrange("p (c w h) -> p c w h", c=CT, w=HW, h=HW)

            # DRAM staging for collectives: rounds {0,1} in one AllGather
            # (hidden under rounds 2-3), then {2} under round 3, then {3}
            OM_SPLIT = ((0, 2), (2, 3), (3, 4))
            om_bi = [dramp.tile([128, CT * (b - a) * CW * HW], BF16, tag=f"om_bi{p}",
                                name=f"om_bi{p}") for p, (a, b) in enumerate(OM_SPLIT)]
            om_bo = [dramp.tile([2, 128, CT * (b - a) * CW * HW], BF16, tag=f"om_bo{p}",
                                name=f"om_bo{p}") for p, (a, b) in enumerate(OM_SPLIT)]

            # ================= S0-S3: LN, transpose, in_proj, conv7, xdown =================
            with tc.tile_pool(name="early", bufs=1) as ep, \
                 tc.tile_pool(name="lnp", bufs=3) as lnp, \
                 tc.tile_pool(name="dw7p", bufs=2) as dw7p, \
                 tc.tile_pool(name="pse", bufs=2, space="PSUM") as pse, \
                 tc.tile_pool(name="xdp", bufs=1, space="PSUM") as xdp:

                xT = ep.tile([128, 6 * P], BF16, tag="xT")       # (ktile, pix)
                x1p = ep.tile([128, CT * 1600], BF16, tag="x1p")  # 40x40 padded
                nc.gpsimd.memset(x1p[:], 0.0)

                # --- S0: LN + PE transpose, phase-batched across the 8 tiles ---
                xh_all = ep.tile([128, 8 * D], F32, tag="xh_all")
                for i in range(8):
                    nc.sync.dma_start(xh_all[:, i * D:(i + 1) * D],
                                      hs[i * 128:(i + 1) * 128, :])
                nc.sync.dma_start(ident_sb[:], ident[:])
                nc.sync.dma_start(w_in_sb[:], w_in[:])
                nc.sync.dma_start(b_in_sb[:], b_in[:])
                nc.sync.dma_start(b7_sb[:], b7[:])
                nc.sync.dma_start(w_xd_sb[:], w_xd[:])
                nc.sync.dma_start(w7v_sb[:], w7v[:])
                stt = ep.tile([128, 8 * 12], F32, tag="stt")
                st3 = stt[:].rearrange("p (i c) -> p i c", i=8)
                agg = ep.tile([128, 8 * 4], F32, tag="agg")
                ag3 = agg[:].rearrange("p (i c) -> p i c", i=8)
                for i in range(8):
                    nc.vector.bn_stats(st3[:, i, 0:6], xh_all[:, i * D:i * D + 384])
                    nc.vector.bn_stats(st3[:, i, 6:12], xh_all[:, i * D + 384:(i + 1) * D])
                    nc.vector.bn_aggr(ag3[:, i, 0:2], st3[:, i, :])
                # rs = 1/sqrt(var+eps), batched over the 8-tile dim
                nc.scalar.activation(ag3[:, :, 2:3], ag3[:, :, 1:2], AF.Ln, bias=cst[:, 0:1])
                nc.scalar.activation(ag3[:, :, 3:4], ag3[:, :, 2:3], AF.Exp, scale=cst[:, 1:2])
                for i in range(8):
                    xhb = lnp.tile([128, D], BF16, tag="xhb")
                    nc.vector.tensor_scalar(xhb[:], xh_all[:, i * D:(i + 1) * D],
                                            ag3[:, i, 0:1], ag3[:, i, 3:4],
                                            op0=OP.subtract, op1=OP.mult)
                    # transpose 6 ktiles -> xT
                    for g in range(2):   # groups of ktiles: 0-3, 4-5
                        kn = 4 if g == 0 else 2
                        pt = pse.tile([128, 512], BF16, tag="tp")
                        for kk in range(kn):
                            k = g * 4 + kk
                            nc.tensor.transpose(pt[:, kk * 128:(kk + 1) * 128],
                                                xhb[:, k * 128:(k + 1) * 128],
                                                ident_sb[:])
                        dst = xT[:].rearrange("p (k t) -> p k t", k=6)[:, g * 4:g * 4 + kn,
                                                                      i * 128:(i + 1) * 128]
                        src = pt[:, 0:kn * 128].rearrange("p (k t) -> p k t", k=kn)
                        nc.scalar.activation(dst, src, AF.Copy)

                # --- S1: in_proj (+folded LN bias) -> x1p interior ---
                x1p4 = x1p[:].rearrange("p (c a b) -> p c a b", c=CT, a=40, b=40)
                for m in range(CT):
                    for nh in range(2):
                        ps = pse.tile([128, 512], F32, tag="ip")
                        for k in range(6):
                            nc.tensor.matmul(ps[:],
                                             w_in_sb[:, k * DH + m * 128:k * DH + (m + 1) * 128],
                                             xT[:, k * P + nh * 512:k * P + (nh + 1) * 512],
                                             start=(k == 0), stop=False)
                        nc.tensor.matmul(ps[:], b_in_sb[:, m * 128:(m + 1) * 128],
                                         ones_sb[:, 0:512], start=False, stop=True)
                        dst = x1p4[:, m, nh * 16 + 4:nh * 16 + 20, 4:36]
                        src = ps[:].rearrange("p (a b) -> p a b", a=16)
                        nc.scalar.activation(dst, src, AF.Copy)

                # --- S2: conv7 split across PE (diag matmuls), Pool + DVE
                # (per-partition-scalar multiply-accumulate in bf16).
                # xdown partials accumulate in PSUM as each ctile finishes so
                # the AllGather can launch immediately after ct2. ---
                NPE = 33   # taps 0-32 on PE; the rest on DVE (bf16 2x)
                xdps = [xdp.tile([48, 512], F32, tag=f"xdps{nh}", name=f"xdps{nh}")
                        for nh in range(2)]
                for ct in range(CT):
                    dwt = dw7p.tile([128, NPE * 128], BF16, tag="dwt")
                    nc.sync.dma_start(dwt[:], dw7[ct, :, 0:NPE * 128])
                    for nh in range(2):
                        ps = pse.tile([128, 512], F32, tag="c7")
                        for tap in range(NPE):
                            dy, dx = tap // 7, tap % 7
                            rhs = x1p4[:, ct, nh * 16 + 1 + dy:nh * 16 + 17 + dy,
                                       1 + dx:33 + dx]
                            nc.tensor.matmul(ps[:], dwt[:, tap * 128:(tap + 1) * 128],
                                             rhs, start=(tap == 0), stop=False)
                        nc.tensor.matmul(ps[:], b7_sb[:, ct * 128:(ct + 1) * 128],
                                         ones_sb[:, 0:512], start=False, stop=True)
                        acc = dw7p.tile([128, 512], BF16, tag="c7acc")
                        acc3 = acc[:].rearrange("p (a b) -> p a b", a=16)
                        nc.vector.memset(acc[:], 0.0)
                        with nc.allow_low_precision(reason="conv7 tail taps bf16"):
                            for tap in range(NPE, 49):
                                dy, dx = tap // 7, tap % 7
                                rhs = x1p4[:, ct, nh * 16 + 1 + dy:nh * 16 + 17 + dy,
                                           1 + dx:33 + dx]
                                nc.vector.scalar_tensor_tensor(
                                    acc3, rhs,
                                    w7v_sb[:, ct * 49 + tap:ct * 49 + tap + 1],
                                    acc3, op0=OP.mult, op1=OP.add)
                        dst = x2h4[:, ct, nh * 16:nh * 16 + 16, :]
                        nc.vector.tensor_add(dst, ps[:].rearrange("p (a b) -> p a b", a=16),
                                             acc3)
                    # transposed copy (w-major), on Pool
                    nc.gpsimd.tensor_copy(x2w4[:, ct], x2h4[:, ct].transpose([0, 2, 1]))
                    # xdown partial accumulation for this ctile
                    for nh in range(2):
                        nc.tensor.matmul(xdps[nh][:],
                                         w_xd_sb[:, ct * 48:(ct + 1) * 48],
                                         x2h[:, ct * P + nh * 512:ct * P + (nh + 1) * 512],
                                         start=(ct == 0), stop=(ct == CT - 1))

                # --- S3: xdown evict (bf16) + AllGather + local add ---
                xp_sb = ep.tile([48, P], BF16, tag="xp_sb")
                for nh in range(2):
                    nc.scalar.activation(xp_sb[:, nh * 512:(nh + 1) * 512],
                                         xdps[nh][:], AF.Copy)
                xp_bi = dramp.tile([48, P], BF16, tag="xp_bi")
                xp_bo = dramp.tile([2, 48, P], BF16, tag="xp_bo")
                nc.sync.dma_start(xp_bi[:], xp_sb[:])
                nc.sync.dma_start(w_g_sb[:], w_g[:])
                nc.sync.dma_start(w_oc_sb[:], w_oc[:])
                nc.sync.dma_start(dw3_sb[:], dw3[:])
                nc.sync.dma_start(w_op_sb[:], w_op[:])
                nc.gpsimd.collective_compute(
                    "AllGather", OP.bypass, replica_groups=REPLICA_PAIRS,
                    ins=[xp_bi.opt()], outs=[xp_bo.opt()])
                xpa = ep.tile([48, P], BF16, tag="xpa")
                xpc = ep.tile([48, P], BF16, tag="xpc")
                nc.sync.dma_start(xpa[:], xp_bo[0])
                nc.sync.dma_start(xpc[:], xp_bo[1])
                nc.vector.tensor_add(xpb[:], xpa[:], xpc[:])

            # ================= S4-S6: gates + scan + merge, per 8-col round =================
            xp3 = xpb[:].rearrange("p (h w) -> p h w", h=HW).transpose([0, 2, 1])
            # xp3: [48, w, h]

            def xs3(d, r):
                v = (x2w4 if d % 2 == 0 else x2h4)[:, 0:3]
                if d >= 2:
                    v = v[:, :, ::-1, :]
                return v[:, :, r * CW:(r + 1) * CW, :]

            def v4(t):
                return t[:].rearrange("p (c w h) -> p c w h", c=SCT, w=CW, h=HW)

            with tc.tile_pool(name="rnd", bufs=2) as rp, \
                 tc.tile_pool(name="psg", bufs=4, space="PSUM") as psg:
                for r in range(NR):
                    gl = rp.tile([128, SCT * CW * HW], BF16, tag="gl")
                    gm = rp.tile([128, SCT * CW * HW], BF16, tag="gm")
                    gr = rp.tile([128, SCT * CW * HW], BF16, tag="gr")
                    uu = rp.tile([128, SCT * CW * HW], BF16, tag="uu")
                    dd = rp.tile([128, SCT * CW * HW], BF16, tag="dd")
                    ss = rp.tile([128, SCT * CW * HW], BF16, tag="ss")
                    rr = rp.tile([128, SCT * CW * HW], BF16, tag="rr")
                    lx = rp.tile([128, SCT * CW * HW], BF16, tag="lx")
                    xd = rp.tile([128, SCT * CW * HW], BF16, tag="xd")

                    gl4, gm4, gr4 = v4(gl), v4(gm), v4(gr)
                    uu4, dd4 = v4(uu), v4(dd)
                    ss4, rr4 = v4(ss), v4(rr)
                    lx4, xd4 = v4(lx), v4(xd)

                    rhs = xp3[:, r * CW:(r + 1) * CW, :]

                    # --- gate preactivations, 3 ctiles per PSUM group ---
                    for ti in range(6):
                        for d in range(4):
                            ps = psg.tile([128, 1024], F32, tag="gp")
                            for j in range(3):
                                m = ti * 12 + d * 3 + j
                                nc.tensor.matmul(ps[:, j * 256:(j + 1) * 256],
                                                 w_g_sb[0:48, m * 128:(m + 1) * 128],
                                                 rhs, start=True, stop=True)
                            ps3 = ps[:, 0:768].rearrange("p (c w h) -> p c w h", c=3, w=CW)
                            if ti == 3:
                                # evict L to SBUF on Act, mul at DVE 2x bf16
                                # (direct PSUM-sourced mul runs at 1x)
                                lp = rp.tile([128, 3 * CW * HW], BF16, tag="lp")
                                lp3 = lp[:].rearrange("p (c w h) -> p c w h",
                                                      c=3, w=CW)
                                nc.scalar.activation(lp3, ps3, AF.Copy)
                                nc.vector.tensor_mul(lx4[:, 3 * d:3 * d + 3], lp3,
                                                     xs3(d, r))
                            else:
                                dstv = (gl4, gm4, gr4, None, uu4, dd4)[ti]
                                fn = AF.Sigmoid if ti < 3 else AF.Copy
                                nc.scalar.activation(dstv[:, 3 * d:3 * d + 3], ps3, fn)
                        if ti == 2:
                            # s = gl+gm+gr with boundary fixes (scan divides by s)
                            nc.vector.tensor_add(ss[:], gl[:], gm[:])
                            nc.vector.tensor_add(ss[:], ss[:], gr[:])
                            nc.vector.tensor_sub(ss4[:, :, :, 0:1], ss4[:, :, :, 0:1],
                                                 gl4[:, :, :, 0:1])
                            nc.vector.tensor_sub(ss4[:, :, :, HW - 1:HW],
                                                 ss4[:, :, :, HW - 1:HW],
                                                 gr4[:, :, :, HW - 1:HW])
                            with nc.allow_low_precision(reason="gate norm bf16"):
                                nc.vector.reciprocal(rr[:], ss[:])
                        elif ti == 5:
                            # xd = D'*xs and dir sum, on Pool
                            for d in range(4):
                                nc.gpsimd.tensor_mul(xd4[:, 3 * d:3 * d + 3],
                                                     dd4[:, 3 * d:3 * d + 3], xs3(d, r))

                    xsum = rp.tile([128, 3 * CW * HW], BF16, tag="xsum")
                    xsum3 = xsum[:].rearrange("p (c w h) -> p c w h", c=3, w=CW)
                    nc.gpsimd.tensor_add(xsum3, xd4[:, 0:3], xd4[:, 3:6])
                    nc.gpsimd.tensor_add(xsum3, xsum3, xd4[:, 6:9])
                    nc.gpsimd.tensor_add(xsum3, xsum3, xd4[:, 9:12])

                    # --- scan (bf16) ---
                    sc = rp.tile([128, SCT * CW * (HW + 2)], BF16, tag="sc")
                    sc4 = sc[:].rearrange("p (c w h) -> p c w h", c=SCT, w=CW, h=HW + 2)
                    nc.gpsimd.memset(sc4[:, :, :, 0:1], 0.0)
                    nc.gpsimd.memset(sc4[:, :, :, HW + 1:HW + 2], 0.0)
                    nc.gpsimd.tensor_copy(sc4[:, :, 0, 1:HW + 1], lx4[:, :, 0, :])
                    for t in range(1, CW):
                        hp = sc4[:, :, t - 1, :]
                        p1 = rp.tile([128, SCT * HW], BF16, tag="p1")
                        p2 = rp.tile([128, SCT * HW], BF16, tag="p2")
                        p13 = p1[:].rearrange("p (c h) -> p c h", c=SCT)
                        p23 = p2[:].rearrange("p (c h) -> p c h", c=SCT)
                        nc.vector.tensor_mul(p13, gl4[:, :, t, :], hp[:, :, 0:HW])
                        nc.vector.tensor_mul(p23, gm4[:, :, t, :], hp[:, :, 1:HW + 1])
                        nc.vector.tensor_add(p13, p13, p23)
                        nc.vector.tensor_mul(p23, gr4[:, :, t, :], hp[:, :, 2:HW + 2])
                        nc.vector.tensor_add(p13, p13, p23)
                        nc.vector.tensor_mul(p13, p13, rr4[:, :, t, :])
                        nc.vector.tensor_add(sc4[:, :, t, 1:HW + 1], p13, lx4[:, :, t, :])

                    # --- merge: out_m = sum_d (sc*U' + xs*D') ---
                    accv = out_m4[:, 0:3, r * CW:(r + 1) * CW, :]
                    pm0 = rp.tile([128, 3 * CW * HW], BF16, tag="pm0")
                    pm03 = pm0[:].rearrange("p (c w h) -> p c w h", c=3, w=CW)
                    nc.gpsimd.tensor_mul(pm03, sc4[:, 0:3, :, 1:HW + 1], uu4[:, 0:3])
                    nc.vector.tensor_add(accv, pm03, xsum3)
                    for d in range(1, 4):
                        pm = rp.tile([128, 3 * CW * HW], BF16, tag="pm")
                        pm3 = pm[:].rearrange("p (c w h) -> p c w h", c=3, w=CW)
                        nc.vector.tensor_mul(pm3, sc4[:, 3 * d:3 * d + 3, :, 1:HW + 1],
                                             uu4[:, 3 * d:3 * d + 3])
                        nc.vector.tensor_add(accv, accv, pm3)

                    # --- ship completed chunks of out_m to the peer ---
                    for p, (a, b) in enumerate(OM_SPLIT):
                        if r == b - 1:
                            nc.sync.dma_start(
                                om_bi[p][:],
                                out_m4[:, 0:3, a * CW:b * CW, :])
                            nc.gpsimd.collective_compute(
                                "AllGather", OP.bypass, replica_groups=REPLICA_PAIRS,
                                ins=[om_bi[p].opt()], outs=[om_bo[p].opt()])

            # ================= S7-S9: tail =================
            with tc.tile_pool(name="tail", bufs=1) as tp2, \
                 tc.tile_pool(name="tevict", bufs=3) as tev, \
                 tc.tile_pool(name="pst", bufs=2, space="PSUM") as pst:

                # --- S7: outconv over all 768 input channels, own 384 outputs ---
                # per-round w-major slabs so each round's AllGather is consumed
                # as soon as it lands
                x_all = tp2.tile([128, 2 * CT * P], BF16, tag="x_all")
                x_all4 = x_all[:].rearrange("p (c w h) -> p c w h", c=2 * CT, w=HW, h=HW)
                oc = tp2.tile([128, CT * P], BF16, tag="oc")    # (ct, w, h)
                oc4 = oc[:].rearrange("p (c w h) -> p c w h", c=CT, w=HW, h=HW)
                # ocp (h-major padded) fills per piece so only the last om
                # piece's transpose-copy sits on the critical tail
                ocp = tp2.tile([128, CT * 34 * 34], BF16, tag="ocp")
                nc.gpsimd.memset(ocp[:], 0.0)
                ocp4 = ocp[:].rearrange("p (c a b) -> p c a b", c=CT, a=34, b=34)
                for p, (a, b) in enumerate(OM_SPLIT):
                    for slab in range(2):
                        nc.sync.dma_start(
                            x_all4[:, slab * CT:(slab + 1) * CT, a * CW:b * CW, :],
                            om_bo[p][slab].rearrange("p (c w h) -> p c w h",
                                                     c=CT, w=(b - a) * CW, h=HW))
                    for w0 in range(a * CW, b * CW, 2 * CW):
                        nw = min(2 * CW, b * CW - w0)
                        for m in range(CT):
                            ps = pst.tile([128, 512], F32, tag="oc")
                            for kk in range(6):
                                rhs = x_all4[:, kk, w0:w0 + nw, :]
                                nc.tensor.matmul(ps[:, 0:nw * HW],
                                                 w_oc_sb[:, kk * DH + m * 128:kk * DH + (m + 1) * 128],
                                                 rhs, start=(kk == 0), stop=(kk == 5))
                            nc.scalar.activation(oc4[:, m, w0:w0 + nw, :],
                                                 ps[:, 0:nw * HW].rearrange("p (w h) -> p w h", w=nw),
                                                 AF.Copy)
                    for ct in range(CT):
                        nc.scalar.activation(
                            ocp4[:, ct, 1:33, 1 + a * CW:1 + b * CW],
                            oc4[:, ct, a * CW:b * CW, :].transpose([0, 2, 1]),
                            AF.Copy)

                # --- S8: outdconv 3x3, relu^2 ---
                # --- S8/S9 interleaved nh-major: outproj of a pixel half starts
                # as soon as that half's dconv3+relu^2 is complete ---
                yy = tp2.tile([128, CT * P], BF16, tag="yy")
                op_bi = dramp.tile([P, D], BF16, tag="op_bi")
                op_bo = dramp.tile([P // 2, D], BF16, tag="op_bo")
                for nh in range(2):
                    for ct in range(CT):
                        ps = pst.tile([128, 512], F32, tag="d3")
                        for tap in range(9):
                            dy, dx = tap // 3, tap % 3
                            rhs = ocp4[:, ct, nh * 16 + dy:nh * 16 + 16 + dy,
                                       dx:32 + dx]
                            nc.tensor.matmul(ps[:],
                                             dw3_sb[:, (ct * 9 + tap) * 128:(ct * 9 + tap + 1) * 128],
                                             rhs, start=(tap == 0), stop=(tap == 8))
                        y0 = tev.tile([128, 512], BF16, tag="y0")
                        nc.scalar.activation(y0[:], ps[:], AF.Relu)
                        with nc.allow_low_precision(reason="relu^2 in bf16"):
                            nc.vector.tensor_mul(
                                yy[:, ct * P + nh * 512:ct * P + (nh + 1) * 512],
                                y0[:], y0[:])
                    for mt in range(nh * 4, nh * 4 + 4):
                        ps = pst.tile([128, 1024], F32, tag="op")
                        for n in range(2):
                            nn = 512 if n == 0 else 256
                            for k in range(CT):
                                nc.tensor.matmul(ps[:, n * 512:n * 512 + nn],
                                                 yy[:, k * P + mt * 128:k * P + (mt + 1) * 128],
                                                 w_op_sb[:, k * D + n * 512:k * D + n * 512 + nn],
                                                 start=(k == 0), stop=(k == CT - 1))
                        oevict = tev.tile([128, D], BF16, tag="oevict")
                        nc.scalar.activation(oevict[:], ps[:, 0:D], AF.Copy)
                        nc.sync.dma_start(op_bi[mt * 128:(mt + 1) * 128, :], oevict[:])
                nc.gpsimd.collective_compute(
                    "ReduceScatter", OP.add, replica_groups=REPLICA_PAIRS,
                    ins=[op_bi.opt()], outs=[op_bo.opt()])
                nc.sync.dma_start(y_out[:], op_bo[:])

    nc.compile()
    return nc


# ======================= host side =======================

def _prep_weights(inputs):
    """Per-half host-folded weight tensors (numpy, bf16)."""
    bf = ml_dtypes.bfloat16
    norm_w = np.asarray(inputs["norm_w"], np.float64)
    norm_b = np.asarray(inputs["norm_b"], np.float64)
    in_proj_w = np.asarray(inputs["in_proj_w"], np.float64)
    conv7_w = np.asarray(inputs["conv7_w"], np.float64)
    conv7_b = np.asarray(inputs["conv7_b"], np.float64)
    xdown_w = np.asarray(inputs["xdown_w"], np.float64)
    wup_w = np.asarray(inputs["wup_w"], np.float64)
    lup_w = np.asarray(inputs["lup_w"], np.float64)
    uup_w = np.asarray(inputs["uup_w"], np.float64)
    dcoef_w = np.asarray(inputs["dcoef_w"], np.float64)
    m_w = np.asarray(inputs["m_w"], np.float64)
    outconv_w = np.asarray(inputs["outconv_w"], np.float64)
    outdconv_w = np.asarray(inputs["outdconv_w"], np.float64)
    outproj_w = np.asarray(inputs["outproj_w"], np.float64)

    Wf = in_proj_w * norm_w[None, :]
    b_in_full = in_proj_w @ norm_b

    idx = np.arange(128)
    out = []
    for half in range(2):
        rows = slice(half * DH, (half + 1) * DH)
        w_in = Wf[rows, :].T.reshape(6, 128, DH).transpose(1, 0, 2).reshape(128, 6 * DH)
        b_in = b_in_full[rows].reshape(1, DH)

        w7 = conv7_w[rows, 0].reshape(CT, 128, 49)           # (ct, k, tap)
        dw7 = np.zeros((CT, 128, 49, 128))
        for ct in range(CT):
            dw7[ct, idx, :, idx] = w7[ct]                    # (128, 49)
        dw7 = dw7.reshape(CT, 128, 49 * 128)
        b7h = conv7_b[rows].reshape(1, DH)

        w_xd = xdown_w[:, rows].T.reshape(CT, 128, 48).transpose(1, 0, 2).reshape(128, CT * 48)
        w7v = w7.reshape(CT, 128, 49).transpose(1, 0, 2).reshape(128, CT * 49)

        # gate weights: [Gl | Gm | Gr | L | U' | D'] each 12 mtiles (dir-major)
        blocks = []
        for ti in range(6):
            for c in range(12):
                k, c3 = c // 3, c % 3
                g0 = k * D + half * DH + c3 * 128
                if ti < 3:
                    src = wup_w[ti * 4 * D + g0: ti * 4 * D + g0 + 128]
                elif ti == 3:
                    src = lup_w[g0:g0 + 128]
                elif ti == 4:
                    src = uup_w[g0:g0 + 128] * m_w[k]
                else:
                    src = dcoef_w[g0:g0 + 128] * m_w[k]
                blocks.append(src)                            # (128, 48)
        wg = np.concatenate(blocks, axis=0).T                 # (48, 9216)
        w_gf = np.zeros((128, 72 * 128))
        w_gf[0:48] = wg

        # outconv: own 384 OUT rows, all 768 input channels in rank order
        w_oc_mat = outconv_w[rows, :].T                       # (768 in, 384 out)
        w_oc = w_oc_mat.reshape(6, 128, DH).transpose(1, 0, 2).reshape(128, 6 * DH)

        w3 = outdconv_w[rows, 0].reshape(CT, 128, 9)
        dw3 = np.zeros((128, CT, 9, 128))
        for ct in range(CT):
            dw3[idx, ct, :, idx] = w3[ct]
        dw3 = dw3.reshape(128, CT * 9 * 128)

        w_op = outproj_w[:, rows].T.reshape(CT, 128, D).transpose(1, 0, 2).reshape(128, CT * D)

        out.append(dict(
            w_in=w_in.astype(bf), b_in=b_in.astype(bf), dw7=dw7.astype(bf),
            b7=b7h.astype(bf), w_xd=w_xd.astype(bf), w7v=w7v.astype(bf),
            w_g=w_gf.astype(bf),
            w_oc=w_oc.astype(bf), dw3=dw3.astype(bf), w_op=w_op.astype(bf),
            ident=np.eye(128).astype(bf),
        ))
    return out


_CACHE = {}


def kernel(**inputs):
    if "nc" not in _CACHE:
        _CACHE["nc"] = build_program()
    nc = _CACHE["nc"]
    in_maps = _make_in_maps(inputs)
    res = bass_utils.run_bass_kernel_spmd(nc, in_maps, core_ids=list(range(8)))
    out = np.empty((4, P, D), np.float32)
    for core in range(8):
        b, half = core // 2, core % 2
        out[b, half * 512:(half + 1) * 512, :] = np.asarray(
            res.results[core]["y"]).astype(np.float32)
    return out, np.asarray(inputs["hidden_states"], np.float32)


def _make_in_maps(inputs):
    halves = _prep_weights(inputs)
    hs_full = np.asarray(inputs["hidden_states"], np.float32)
    in_maps = []
    for core in range(8):
        b, half = core // 2, core % 2
        m = dict(halves[half])
        m["hs"] = np.ascontiguousarray(hs_full[b])
        in_maps.append(m)
    return in_maps


def bench(inputs, iters=20):
    """Device-resident repeated execution; returns median per-call wall ns."""
    import time
    import jax
    from jax.experimental.shard_map import shard_map
    from jax.sharding import Mesh, PartitionSpec
    from concourse import bass2jax, mybir as _mb

    if "nc" not in _CACHE:
        _CACHE["nc"] = build_program()
    nc = _CACHE["nc"]
    in_maps = _make_in_maps(inputs)
    bass2jax.install_neuronx_cc_hook()

    n_cores = 8
    in_names, out_names, out_avals, zero_outs = [], [], [], []
    partition_name = nc.partition_id_tensor.name if nc.partition_id_tensor else None
    for alloc in nc.m.functions[0].allocations:
        if not isinstance(alloc, _mb.MemoryLocationSet):
            continue
        name = alloc.memorylocations[0].name
        if alloc.kind == "ExternalInput":
            if name != partition_name:
                in_names.append(name)
        elif alloc.kind == "ExternalOutput":
            shape = tuple(alloc.tensor_shape)
            dtype = _mb.dt.np(alloc.dtype)
            out_names.append(name)
            out_avals.append(jax.core.ShapedArray(shape, dtype))
            zero_outs.append(np.zeros(shape, dtype))
    n_params = len(in_names)
    all_in_names = list(in_names) + list(out_names)
    if partition_name is not None:
        all_in_names.append(partition_name)

    import jax.numpy as jnp
    hs_idx = in_names.index("hs")

    def _make_body(reps):
        def _body(*args):
            operands = list(args)
            outs = None
            for _ in range(reps):
                ops = list(operands)
                if partition_name is not None:
                    ops.append(bass2jax.partition_id_tensor())
                outs = bass2jax._bass_exec_p.bind(
                    *ops, out_avals=tuple(out_avals), in_names=tuple(all_in_names),
                    out_names=tuple(out_names), lowering_input_output_aliases=(),
                    sim_require_finite=True, sim_require_nnan=True, nc=nc)
                y = outs[0]
                pad = jnp.concatenate([y, y], axis=0).astype(jnp.float32)
                operands[hs_idx] = operands[hs_idx] + 0.0 * pad
            return tuple(outs)
        return _body
    _body = _make_body(1)

    devices = jax.devices()[:n_cores]
    mesh = Mesh(np.asarray(devices), ("core",))
    nspec = (PartitionSpec("core"),) * (n_params + len(out_names))
    concat_in = [np.concatenate([np.asarray(in_maps[c][nm]) for c in range(n_cores)], axis=0)
                 for nm in in_names]
    concat_zero = [np.zeros((n_cores * z.shape[0], *z.shape[1:]), z.dtype) for z in zero_outs]
    sharding = jax.sharding.NamedSharding(mesh, PartitionSpec("core"))
    dev_args = [jax.device_put(a, sharding) for a in concat_in + concat_zero]

    def timed(reps, n):
        fn = jax.jit(shard_map(_make_body(reps), mesh=mesh, in_specs=nspec,
                               out_specs=(PartitionSpec("core"),) * len(out_names),
                               check_rep=False), keep_unused=True)
        r = fn(*dev_args)
        jax.block_until_ready(r)
        ts = []
        for _ in range(n):
            t0 = time.perf_counter()
            r = fn(*dev_args)
            jax.block_until_ready(r)
            ts.append(time.perf_counter() - t0)
        ts.sort()
        return ts[len(ts) // 2]

    t1 = timed(1, iters)
    print(f"[bench] per-call wall (incl ~70ms axon dispatch floor): {t1*1e3:.2f}ms")
    return t1 * 1e9

